# revision 6
# baseline (speedup 1.0000x reference)
"""ChebConv (K=4) Trainium2 Bass kernel, v2.

Problem (hardcoded): B=16 graphs, N=2048 nodes, F=64 feats, K=4, out_dim=128.
  L = D A0 D  (A0 = A with zeroed diag, D = diag(1/(eps+sqrt(rowsum(A0)))))
  T0 = X; T1 = L X; T_t = 2 L T_{t-1} - T_{t-2}
  out = relu(concat(T0..T3) @ kernel + bias)

Sharding: batch across 8 cores, 2 graphs per core; host concatenates outputs.

Key layout decision: the host stages A TRANSPOSED (per graph, A[g].T, f32).
A loaded chunk then holds At[j in chunk, i] with the contraction index j on
partitions, so it is directly usable as the PE's stationary operand:
    out[i-block, f] += sum_j A[i, j] * Z[j, f]
with the Z chunk (64 cols) as the moving operand. This removes the 512
PE transposes + their PSUM drains of the previous version and halves the
Chebyshev matmul cost (128-wide stationary instead of 64).

Per core (graphs g=0,1), with Z_t := d * T_t:
  d from rowsums: column-sums of At chunks via 1-col ones matmuls on PE
  Z0      = d*X
  Z_t     = 2 d^2 * (A0 @ Z_{t-1}) - Z_{t-2}     (node-major throughout)
  out     = relu( (1/d) * (sum_t Z_t @ K_t) + bias )
The (1/d) row scale is folded into the e-scaled Z^T tiles used by the
projection (via a diag(e) moving operand in the transpose matmuls).

A arrives f32 in HBM; SWDGE casts to bf16 in flight, 4 chunks per dma_start
(descriptor-generation on Pool is the serial cost; transfers overlap).
Graph 0's compute overlaps graph 1's load.
"""

import numpy as np

P = 128          # partitions
N = 2048         # nodes per graph
F = 64           # input features
KORD = 4         # Chebyshev order
OUT = 128        # output features
GP = 2           # graphs per core
NT = N // P      # 16 node chunks
NCORES = 8
CPD = 4          # chunks per A dma_start
NDMA = NT // CPD # A dma_starts per graph

_cached = {}


def _build_nc():
    import ml_dtypes
    import concourse.bacc as bacc
    import concourse.mybir as mybir
    from concourse.tile import TileContext

    f32 = mybir.dt.float32
    bf16 = mybir.dt.bfloat16
    Alu = mybir.AluOpType
    Act = mybir.ActivationFunctionType

    nc = bacc.Bacc("TRN2", target_bir_lowering=False)

    a_in = nc.dram_tensor("a", [GP, N, N], f32, kind="ExternalInput")  # A^T!
    x_in = nc.dram_tensor("x", [GP, N, F], f32, kind="ExternalInput")
    wk_in = nc.dram_tensor("wk", [KORD * F, OUT], f32, kind="ExternalInput")
    bias_in = nc.dram_tensor("bias", [OUT], f32, kind="ExternalInput")
    o_out = nc.dram_tensor("out", [GP, N, OUT], f32, kind="ExternalOutput")

    ident_np = np.eye(P, dtype=ml_dtypes.bfloat16)
    ident_dram = nc.inline_tensor(ident_np, name="identbf")

    with TileContext(nc) as tc, tc.tile_pool(name="const", bufs=1) as const, \
         tc.tile_pool(name="big", bufs=1) as big, \
         tc.tile_pool(name="small", bufs=1) as small, \
         tc.tile_pool(name="stage", bufs=2) as stage, \
         tc.tile_pool(name="outs", bufs=2) as outs, \
         tc.tile_pool(name="ps_rs", bufs=1, space="PSUM") as ps_rs, \
         tc.tile_pool(name="ps_it", bufs=4, space="PSUM") as ps_it, \
         tc.tile_pool(name="ps_ms", bufs=2, space="PSUM") as ps_ms:

        # ---- constants -------------------------------------------------
        ident = const.tile([P, P], bf16)
        nc.sync.dma_start(out=ident, in_=ident_dram[:, :])
        # mask = 1 - I  (bf16), for in-place diag zeroing
        mask = const.tile([P, P], bf16)
        nc.vector.tensor_scalar(mask, ident, -1.0, 1.0, Alu.mult, Alu.add)
        # ones column (rowsum moving operand)
        ones_col = const.tile([P, 1], bf16)
        nc.vector.memset(ones_col, 1.0)
        # kernel: [256,128] f32 -> two bf16 tiles [128,128] (t-pairs 01, 23)
        kab = const.tile([P, OUT], bf16)
        kcd = const.tile([P, OUT], bf16)
        kstage = stage.tile([P, 2 * OUT], f32, name="kstage")
        nc.sync.dma_start(out=kstage[:, 0:OUT], in_=wk_in[0:P, :])
        nc.sync.dma_start(out=kstage[:, OUT : 2 * OUT], in_=wk_in[P : 2 * P, :])
        nc.scalar.copy(out=kab, in_=kstage[:, 0:OUT])
        nc.scalar.copy(out=kcd, in_=kstage[:, OUT : 2 * OUT])
        # bias row [1,128] bf16 + ones row [1,128] bf16 (for bias matmul)
        bias_row = const.tile([1, OUT], bf16)
        bias_f32 = const.tile([1, OUT], f32)
        nc.sync.dma_start(out=bias_f32, in_=bias_in[None, :])
        nc.scalar.copy(out=bias_row, in_=bias_f32)
        ones_row = const.tile([1, P], bf16)
        nc.vector.memset(ones_row, 1.0)

        # ---- persistent SBUF state ------------------------------------
        # A^T per graph: [:, q, :] holds rows j = 128q+p, free = node i
        at = [big.tile([P, NT, N], bf16, name=f"at{g}") for g in range(GP)]
        # Z tiles: zja = (Z0 | Z1), zjb = (Z2 | Z3), per graph, node-major
        zja = [big.tile([P, NT, 2 * F], bf16, name=f"zja{g}") for g in range(GP)]
        zjb = [big.tile([P, NT, 2 * F], bf16, name=f"zjb{g}") for g in range(GP)]
        # X staging (f32) per graph
        xst = [big.tile([P, NT, F], f32, name=f"xst{g}") for g in range(GP)]
        # e-scaled Z^T pairs per graph: rows 0:64 = even t, 64:128 = odd t
        ztab = [big.tile([P, N], bf16, name=f"ztab{g}") for g in range(GP)]
        ztcd = [big.tile([P, N], bf16, name=f"ztcd{g}") for g in range(GP)]
        # diag(e) moving operands for the Z^T builds, one [128,128] per chunk
        dge = [big.tile([P, NT, P], bf16, name=f"dge{g}") for g in range(GP)]
        # row stats per graph [128, NT] f32 (col = node chunk)
        rs = [small.tile([P, NT], f32, name=f"rs{g}") for g in range(GP)]
        dd = [small.tile([P, NT], f32, name=f"dd{g}") for g in range(GP)]   # d/2
        d2s = [small.tile([P, NT], f32, name=f"d2s{g}") for g in range(GP)] # 2d^2
        evec = [small.tile([P, NT], f32, name=f"ev{g}") for g in range(GP)] # 1/d
        tch = [small.tile([P, NT], f32, name=f"tch{g}") for g in range(GP)]
        uch = [small.tile([P, NT], f32, name=f"uch{g}") for g in range(GP)]
        wch = [small.tile([P, NT], f32, name=f"wch{g}") for g in range(GP)]

        def zslice(g, t, q):
            # bf16 Z_t chunk q, [128, 64]
            base = zja[g] if t < 2 else zjb[g]
            h = t % 2
            return base[:, q, h * F : (h + 1) * F]

        # rowsum PSUM: one tile, 16 cols per graph
        rsps = ps_rs.tile([P, GP * NT], f32, name="rsps")

        # ---- X loads (SP queue, f32) ----------------------------------
        for g in range(GP):
            nc.sync.dma_start(
                out=xst[g],
                in_=x_in[g].rearrange("(c p) f -> p c f", p=P),
            )

        # ---- A loads: SWDGE cast f32->bf16, CPD chunks per dma --------
        for g in range(GP):
            for d0 in range(NDMA):
                c0 = d0 * CPD
                nc.gpsimd.dma_start(
                    out=at[g][:, c0 : c0 + CPD, :],
                    in_=a_in[g, c0 * P : (c0 + CPD) * P, :].rearrange(
                        "(c p) n -> p c n", p=P
                    ),
                )

        def prep(g, zd_engine):
            # zero diag blocks (chase chunk arrivals)
            for q in range(NT):
                blk = at[g][:, q, q * P : (q + 1) * P]
                zd_engine.scalar_tensor_tensor(blk, blk, 1.0, mask, Alu.mult, Alu.mult)
            # rowsums: d_i = sum_j A0[i,j] via 1-col ones matmuls. Groups must
            # be sequential per psum tile (interleaved groups accumulate
            # wrongly), so emit b-major after all chunks are resident.
            for b in range(NT):
                col = g * NT + b
                for q in range(NT):
                    nc.tensor.matmul(
                        rsps[:, col : col + 1],
                        lhsT=at[g][:, q, b * P : (b + 1) * P],
                        rhs=ones_col,
                        start=(q == 0), stop=(q == NT - 1),
                        skip_group_check=True,
                    )
            nc.vector.tensor_copy(rs[g], rsps[:, g * NT : (g + 1) * NT])
            # d chain (Newton-refined sqrt: w = rs/sqrt(rs) + sqrt(rs) = 2 sqrt)
            nc.scalar.activation(tch[g], rs[g], Act.Sqrt)
            nc.vector.reciprocal(uch[g], tch[g])
            nc.vector.scalar_tensor_tensor(uch[g], uch[g], 1.0, rs[g], Alu.mult, Alu.mult)
            nc.vector.scalar_tensor_tensor(wch[g], uch[g], 1.0, tch[g], Alu.mult, Alu.add)
            # dd = 1/w = d/2 ; e = w/2 = 1/d ; d2s = 8*dd^2 = 2 d^2
            nc.vector.reciprocal(dd[g], wch[g])
            nc.vector.tensor_scalar_mul(evec[g], wch[g], 0.5)
            nc.vector.scalar_tensor_tensor(d2s[g], dd[g], 8.0, dd[g], Alu.mult, Alu.mult)
            # Z0 = d*X = (X*dd)*2
            for c in range(NT):
                nc.vector.tensor_scalar(
                    zslice(g, 0, c), xst[g][:, c, :],
                    dd[g][:, c : c + 1], 2.0, Alu.mult, Alu.mult,
                )
            # diag(e) moving operands (needed from ztpair builds on)
            for c in range(NT):
                nc.vector.tensor_scalar_mul(
                    dge[g][:, c, :], ident, evec[g][:, c : c + 1]
                )

        def iteration(g, t):
            # Z_t = 2 d^2 (A0 @ Z_{t-1}) - Z_{t-2}, 16 i-blocks, psum in halves
            for half in range(2):
                ps = ps_it.tile([P, 512], f32, name="psit", tag="psit")
                for bb in range(8):
                    b = half * 8 + bb
                    reg = ps[:, bb * F : (bb + 1) * F]
                    for q in range(NT):
                        nc.tensor.matmul(
                            reg,
                            lhsT=at[g][:, q, b * P : (b + 1) * P],
                            rhs=zslice(g, t - 1, q),
                            start=(q == 0), stop=(q == NT - 1),
                            skip_group_check=True,
                        )
                    zdst = zslice(g, t, b)
                    if t == 1:
                        # Z1 = d^2 W = (W * d2s) * 0.5
                        nc.vector.tensor_scalar(
                            zdst, reg, d2s[g][:, b : b + 1], 0.5,
                            Alu.mult, Alu.mult,
                        )
                    else:
                        nc.vector.scalar_tensor_tensor(
                            zdst, reg, d2s[g][:, b : b + 1],
                            zslice(g, t - 2, b), Alu.mult, Alu.subtract,
                        )

        def ztpair(g, pair):
            # e-scaled transposed copies: ztab (T0|T1) or ztcd (T2|T3)
            src = zja[g] if pair == 0 else zjb[g]
            dst = ztab[g] if pair == 0 else ztcd[g]
            for s in range(4):
                psz = ps_ms.tile([P, 512], f32, name="psz", tag="psms")
                for j in range(4):
                    c = 4 * s + j
                    nc.tensor.matmul(
                        psz[:, j * P : (j + 1) * P],
                        lhsT=src[:, c, :],
                        rhs=dge[g][:, c, :],
                        start=True, stop=True,
                        skip_group_check=True,
                    )
                nc.scalar.copy(out=dst[:, s * 512 : (s + 1) * 512], in_=psz)

        def proj(g):
            for s in range(4):
                pso = ps_ms.tile([P, 512], f32, name="pso", tag="psms")
                for j in range(4):
                    b = 4 * s + j
                    sl = pso[:, j * OUT : (j + 1) * OUT]
                    nc.tensor.matmul(
                        sl, lhsT=ztab[g][:, b * P : (b + 1) * P], rhs=kab,
                        start=True, stop=False, skip_group_check=True,
                    )
                    nc.tensor.matmul(
                        sl, lhsT=ztcd[g][:, b * P : (b + 1) * P], rhs=kcd,
                        start=False, stop=False, skip_group_check=True,
                    )
                    nc.tensor.matmul(
                        sl, lhsT=ones_row, rhs=bias_row,
                        start=False, stop=True, skip_group_check=True,
                    )
                ot = outs.tile([P, 4, OUT], f32, name="ot", tag="ot")
                nc.scalar.activation(ot, pso, Act.Relu)
                nc.sync.dma_start(
                    out=o_out[g, s * 512 : (s + 1) * 512, :].rearrange(
                        "(j p) o -> p j o", p=P
                    ),
                    in_=ot,
                )

        # ---- schedule: g0 compute overlaps g1 load --------------------
        prep(0, nc.vector)
        iteration(0, 1)
        prep(1, nc.vector)   # g1 chunks have all arrived by the time DVE gets here
        ztpair(0, 0)
        iteration(0, 2)
        iteration(0, 3)
        ztpair(0, 1)
        proj(0)
        iteration(1, 1)
        ztpair(1, 0)
        iteration(1, 2)
        iteration(1, 3)
        ztpair(1, 1)
        proj(1)

    nc.finalize()
    return nc


def _get_nc():
    if "nc" not in _cached:
        _cached["nc"] = _build_nc()
    return _cached["nc"]


def kernel(X, A, kernel, bias):
    from concourse.bass_utils import run_bass_kernel_spmd

    nc = _get_nc()
    wk = np.ascontiguousarray(np.asarray(kernel, dtype=np.float32))
    bs = np.ascontiguousarray(np.asarray(bias, dtype=np.float32))
    A = np.asarray(A, dtype=np.float32)
    X = np.asarray(X, dtype=np.float32)
    in_maps = [
        {
            # stage A transposed: device contracts over partitions = A columns
            "a": np.ascontiguousarray(
                A[GP * c : GP * (c + 1)].transpose(0, 2, 1)
            ),
            "x": np.ascontiguousarray(X[GP * c : GP * (c + 1)]),
            "wk": wk,
            "bias": bs,
        }
        for c in range(NCORES)
    ]
    res = run_bass_kernel_spmd(nc, in_maps, core_ids=list(range(NCORES)))
    return np.concatenate([r["out"] for r in res.results], axis=0)


# revision 10
# speedup vs baseline: 1.1588x; 1.1588x over previous
"""ChebConv (K=4) Trainium2 Bass kernel, v2.

Problem (hardcoded): B=16 graphs, N=2048 nodes, F=64 feats, K=4, out_dim=128.
  L = D A0 D  (A0 = A with zeroed diag, D = diag(1/(eps+sqrt(rowsum(A0)))))
  T0 = X; T1 = L X; T_t = 2 L T_{t-1} - T_{t-2}
  out = relu(concat(T0..T3) @ kernel + bias)

Sharding: batch across 8 cores, 2 graphs per core; host concatenates outputs.

Key layout decision: the host stages A TRANSPOSED (per graph, A[g].T, f32).
A loaded chunk then holds At[j in chunk, i] with the contraction index j on
partitions, so it is directly usable as the PE's stationary operand:
    out[i-block, f] += sum_j A[i, j] * Z[j, f]
with the Z chunk (64 cols) as the moving operand. This removes the 512
PE transposes + their PSUM drains of the previous version and halves the
Chebyshev matmul cost (128-wide stationary instead of 64).

Per core (graphs g=0,1), with Z_t := d * T_t:
  d from rowsums: column-sums of At chunks via 1-col ones matmuls on PE
  Z0      = d*X
  Z_t     = 2 d^2 * (A0 @ Z_{t-1}) - Z_{t-2}     (node-major throughout)
  out     = relu( (1/d) * (sum_t Z_t @ K_t) + bias )
The (1/d) row scale is folded into the e-scaled Z^T tiles used by the
projection (via a diag(e) moving operand in the transpose matmuls).

A arrives f32 in HBM; SWDGE casts to bf16 in flight, 4 chunks per dma_start
(descriptor-generation on Pool is the serial cost; transfers overlap).
Graph 0's compute overlaps graph 1's load.
"""

import numpy as np

P = 128          # partitions
N = 2048         # nodes per graph
F = 64           # input features
KORD = 4         # Chebyshev order
OUT = 128        # output features
GP = 2           # graphs per core
NT = N // P      # 16 node chunks
NCORES = 8
CPD = 4          # chunks per A dma_start
NDMA = NT // CPD # A dma_starts per graph

_cached = {}


def _build_nc():
    import ml_dtypes
    import concourse.bacc as bacc
    import concourse.mybir as mybir
    from concourse.tile import TileContext

    f32 = mybir.dt.float32
    bf16 = mybir.dt.bfloat16
    Alu = mybir.AluOpType
    Act = mybir.ActivationFunctionType

    nc = bacc.Bacc("TRN2", target_bir_lowering=False)

    # A^T, pre-cast to bf16 on the host: lets all three DMA queues (SP/ACT
    # HWDGE + Pool SWDGE) load it at the same per-byte rate with no casts.
    a_in = nc.dram_tensor("a", [GP, N, N], bf16, kind="ExternalInput")
    x_in = nc.dram_tensor("x", [GP, N, F], f32, kind="ExternalInput")
    wk_in = nc.dram_tensor("wk", [KORD * F, OUT], f32, kind="ExternalInput")
    bias_in = nc.dram_tensor("bias", [OUT], f32, kind="ExternalInput")
    o_out = nc.dram_tensor("out", [GP, N, OUT], f32, kind="ExternalOutput")

    ident_np = np.eye(P, dtype=ml_dtypes.bfloat16)
    ident_dram = nc.inline_tensor(ident_np, name="identbf")

    with TileContext(nc) as tc, tc.tile_pool(name="const", bufs=1) as const, \
         tc.tile_pool(name="big", bufs=1) as big, \
         tc.tile_pool(name="small", bufs=1) as small, \
         tc.tile_pool(name="stage", bufs=2) as stage, \
         tc.tile_pool(name="outs", bufs=2) as outs, \
         tc.tile_pool(name="ps_rs", bufs=1, space="PSUM") as ps_rs, \
         tc.tile_pool(name="ps_it", bufs=5, space="PSUM") as ps_it, \
         tc.tile_pool(name="ps_ms", bufs=2, space="PSUM") as ps_ms:

        # ---- constants -------------------------------------------------
        ident = const.tile([P, P], bf16)
        nc.sync.dma_start(out=ident, in_=ident_dram[:, :])
        # mask = 1 - I  (bf16), for in-place diag zeroing
        mask = const.tile([P, P], bf16)
        nc.vector.tensor_scalar(mask, ident, -1.0, 1.0, Alu.mult, Alu.add)
        # ones column (rowsum moving operand)
        ones_col = const.tile([P, 1], bf16)
        nc.vector.memset(ones_col, 1.0)
        # kernel: [256,128] f32 -> two bf16 tiles [128,128] (t-pairs 01, 23)
        kab = const.tile([P, OUT], bf16)
        kcd = const.tile([P, OUT], bf16)
        kstage = stage.tile([P, 2 * OUT], f32, name="kstage")
        nc.sync.dma_start(out=kstage[:, 0:OUT], in_=wk_in[0:P, :])
        nc.sync.dma_start(out=kstage[:, OUT : 2 * OUT], in_=wk_in[P : 2 * P, :])
        nc.scalar.copy(out=kab, in_=kstage[:, 0:OUT])
        nc.scalar.copy(out=kcd, in_=kstage[:, OUT : 2 * OUT])
        # bias row [1,128] bf16 + ones row [1,128] bf16 (for bias matmul)
        bias_row = const.tile([1, OUT], bf16)
        bias_f32 = const.tile([1, OUT], f32)
        nc.sync.dma_start(out=bias_f32, in_=bias_in[None, :])
        nc.scalar.copy(out=bias_row, in_=bias_f32)
        ones_row = const.tile([1, P], bf16)
        nc.vector.memset(ones_row, 1.0)

        # ---- persistent SBUF state ------------------------------------
        # A^T per graph: [:, q, :] holds rows j = 128q+p, free = node i
        at = [big.tile([P, NT, N], bf16, name=f"at{g}") for g in range(GP)]
        # Z tiles: zja = (Z0 | Z1), zjb = (Z2 | Z3), per graph, node-major
        zja = [big.tile([P, NT, 2 * F], bf16, name=f"zja{g}") for g in range(GP)]
        zjb = [big.tile([P, NT, 2 * F], bf16, name=f"zjb{g}") for g in range(GP)]
        # X staging (f32) per graph
        xst = [big.tile([P, NT, F], f32, name=f"xst{g}") for g in range(GP)]
        # e-scaled Z^T pairs per graph: rows 0:64 = even t, 64:128 = odd t
        ztab = [big.tile([P, N], bf16, name=f"ztab{g}") for g in range(GP)]
        ztcd = [big.tile([P, N], bf16, name=f"ztcd{g}") for g in range(GP)]
        # diag(e) moving operands for the Z^T builds, one [128,128] per chunk
        dge = [big.tile([P, NT, P], bf16, name=f"dge{g}") for g in range(GP)]
        # row stats per graph [128, NT] f32 (col = node chunk)
        rs = [small.tile([P, NT], f32, name=f"rs{g}") for g in range(GP)]
        dd = [small.tile([P, NT], f32, name=f"dd{g}") for g in range(GP)]   # d/2
        d2s = [small.tile([P, NT], f32, name=f"d2s{g}") for g in range(GP)] # 2d^2
        evec = [small.tile([P, NT], f32, name=f"ev{g}") for g in range(GP)] # 1/d
        tch = [small.tile([P, NT], f32, name=f"tch{g}") for g in range(GP)]
        uch = [small.tile([P, NT], f32, name=f"uch{g}") for g in range(GP)]
        wch = [small.tile([P, NT], f32, name=f"wch{g}") for g in range(GP)]

        def zslice(g, t, q):
            # bf16 Z_t chunk q, [128, 64]
            base = zja[g] if t < 2 else zjb[g]
            h = t % 2
            return base[:, q, h * F : (h + 1) * F]

        # rowsum PSUM: one tile, 16 cols per graph
        rsps = ps_rs.tile([P, GP * NT], f32, name="rsps")

        # ---- X loads (SP queue, f32) ----------------------------------
        for g in range(GP):
            nc.sync.dma_start(
                out=xst[g],
                in_=x_in[g].rearrange("(c p) f -> p c f", p=P),
            )

        # ---- A loads: bf16 chunks round-robined over 3 DMA queues -----
        # (modeled queue-held time ~ bytes/partition; queues run parallel)
        lanes = [nc.sync, nc.scalar, nc.gpsimd]
        asgn = [0, 0, 0, 0, 0, 1, 1, 1, 1, 1, 2, 2, 2, 2, 2, 2]  # SP:5 ACT:5 Pool:6
        for g in range(GP):
            for lane in range(3):
                chunks = [q for q in range(NT) if asgn[q] == lane]
                for i0 in range(0, len(chunks), CPD):
                    grp = chunks[i0 : i0 + CPD]
                    # grouped chunks are consecutive by construction
                    c0, cn = grp[0], len(grp)
                    lanes[lane].dma_start(
                        out=at[g][:, c0 : c0 + cn, :],
                        in_=a_in[g, c0 * P : (c0 + cn) * P, :].rearrange(
                            "(c p) n -> p c n", p=P
                        ),
                    )

        def prep(g, zd_engine):
            # zero diag blocks (chase chunk arrivals)
            for q in range(NT):
                blk = at[g][:, q, q * P : (q + 1) * P]
                zd_engine.scalar_tensor_tensor(blk, blk, 1.0, mask, Alu.mult, Alu.mult)
            # rowsums: d_i = sum_j A0[i,j] via 1-col ones matmuls. Groups must
            # be sequential per psum tile (interleaved groups accumulate
            # wrongly), so emit b-major after all chunks are resident.
            for b in range(NT):
                col = g * NT + b
                for q in range(NT):
                    nc.tensor.matmul(
                        rsps[:, col : col + 1],
                        lhsT=at[g][:, q, b * P : (b + 1) * P],
                        rhs=ones_col,
                        start=(q == 0), stop=(q == NT - 1),
                        skip_group_check=True,
                    )
            nc.vector.tensor_copy(rs[g], rsps[:, g * NT : (g + 1) * NT])
            # d chain (Newton-refined sqrt: w = rs/sqrt(rs) + sqrt(rs) = 2 sqrt)
            nc.scalar.activation(tch[g], rs[g], Act.Sqrt)
            nc.vector.reciprocal(uch[g], tch[g])
            nc.vector.scalar_tensor_tensor(uch[g], uch[g], 1.0, rs[g], Alu.mult, Alu.mult)
            nc.vector.scalar_tensor_tensor(wch[g], uch[g], 1.0, tch[g], Alu.mult, Alu.add)
            # dd = 1/w = d/2 ; e = w/2 = 1/d ; d2s = 8*dd^2 = 2 d^2
            nc.vector.reciprocal(dd[g], wch[g])
            nc.vector.tensor_scalar_mul(evec[g], wch[g], 0.5)
            nc.vector.scalar_tensor_tensor(d2s[g], dd[g], 8.0, dd[g], Alu.mult, Alu.mult)
            # Z0 = d*X = (X*dd)*2
            for c in range(NT):
                nc.vector.tensor_scalar(
                    zslice(g, 0, c), xst[g][:, c, :],
                    dd[g][:, c : c + 1], 2.0, Alu.mult, Alu.mult,
                )
            # diag(e) moving operands (needed from ztpair builds on)
            for c in range(NT):
                nc.vector.tensor_scalar_mul(
                    dge[g][:, c, :], ident, evec[g][:, c : c + 1]
                )

        def iteration(g, t):
            # Z_t = 2 d^2 (A0 @ Z_{t-1}) - Z_{t-2}, 16 i-blocks, psum in halves
            for half in range(2):
                ps = ps_it.tile([P, 512], f32, name="psit", tag="psit")
                for bb in range(8):
                    b = half * 8 + bb
                    reg = ps[:, bb * F : (bb + 1) * F]
                    for q in range(NT):
                        nc.tensor.matmul(
                            reg,
                            lhsT=at[g][:, q, b * P : (b + 1) * P],
                            rhs=zslice(g, t - 1, q),
                            start=(q == 0), stop=(q == NT - 1),
                            skip_group_check=True,
                        )
                    zdst = zslice(g, t, b)
                    if t == 1:
                        # Z1 = d^2 W = (W * d2s) * 0.5
                        nc.vector.tensor_scalar(
                            zdst, reg, d2s[g][:, b : b + 1], 0.5,
                            Alu.mult, Alu.mult,
                        )
                    else:
                        nc.vector.scalar_tensor_tensor(
                            zdst, reg, d2s[g][:, b : b + 1],
                            zslice(g, t - 2, b), Alu.mult, Alu.subtract,
                        )

        def ztpair(g, pair):
            # e-scaled transposed copies: ztab (T0|T1) or ztcd (T2|T3)
            src = zja[g] if pair == 0 else zjb[g]
            dst = ztab[g] if pair == 0 else ztcd[g]
            for s in range(4):
                psz = ps_ms.tile([P, 512], f32, name="psz", tag="psms")
                for j in range(4):
                    c = 4 * s + j
                    nc.tensor.matmul(
                        psz[:, j * P : (j + 1) * P],
                        lhsT=src[:, c, :],
                        rhs=dge[g][:, c, :],
                        start=True, stop=True,
                        skip_group_check=True,
                    )
                nc.scalar.copy(out=dst[:, s * 512 : (s + 1) * 512], in_=psz)

        def proj(g):
            for s in range(4):
                pso = ps_ms.tile([P, 512], f32, name="pso", tag="psms")
                for j in range(4):
                    b = 4 * s + j
                    sl = pso[:, j * OUT : (j + 1) * OUT]
                    nc.tensor.matmul(
                        sl, lhsT=ztab[g][:, b * P : (b + 1) * P], rhs=kab,
                        start=True, stop=False, skip_group_check=True,
                    )
                    nc.tensor.matmul(
                        sl, lhsT=ztcd[g][:, b * P : (b + 1) * P], rhs=kcd,
                        start=False, stop=False, skip_group_check=True,
                    )
                    nc.tensor.matmul(
                        sl, lhsT=ones_row, rhs=bias_row,
                        start=False, stop=True, skip_group_check=True,
                    )
                ot = outs.tile([P, 4, OUT], f32, name="ot", tag="ot")
                nc.scalar.activation(ot, pso, Act.Relu)
                nc.sync.dma_start(
                    out=o_out[g, s * 512 : (s + 1) * 512, :].rearrange(
                        "(j p) o -> p j o", p=P
                    ),
                    in_=ot,
                )

        # ---- schedule: g0 compute overlaps g1 load --------------------
        prep(0, nc.vector)
        iteration(0, 1)
        prep(1, nc.vector)   # g1 chunks have all arrived by the time DVE gets here
        ztpair(0, 0)
        iteration(0, 2)
        iteration(0, 3)
        ztpair(0, 1)
        proj(0)
        iteration(1, 1)
        ztpair(1, 0)
        iteration(1, 2)
        iteration(1, 3)
        ztpair(1, 1)
        proj(1)

    nc.finalize()
    return nc


def _get_nc():
    if "nc" not in _cached:
        _cached["nc"] = _build_nc()
    return _cached["nc"]


def kernel(X, A, kernel, bias):
    import ml_dtypes
    from concourse.bass_utils import run_bass_kernel_spmd

    nc = _get_nc()
    wk = np.ascontiguousarray(np.asarray(kernel, dtype=np.float32))
    bs = np.ascontiguousarray(np.asarray(bias, dtype=np.float32))
    A = np.asarray(A, dtype=np.float32)
    X = np.asarray(X, dtype=np.float32)
    in_maps = [
        {
            # stage A transposed (device contracts over partitions = A
            # columns) and bf16 (the dtype the device kernel computes in)
            "a": np.ascontiguousarray(
                A[GP * c : GP * (c + 1)].transpose(0, 2, 1)
            ).astype(ml_dtypes.bfloat16),
            "x": np.ascontiguousarray(X[GP * c : GP * (c + 1)]),
            "wk": wk,
            "bias": bs,
        }
        for c in range(NCORES)
    ]
    res = run_bass_kernel_spmd(nc, in_maps, core_ids=list(range(NCORES)))
    return np.concatenate([r["out"] for r in res.results], axis=0)


# revision 11
# speedup vs baseline: 1.8370x; 1.5853x over previous
"""ChebConv (K=4) Trainium2 Bass kernel, v3 (fp8).

Problem (hardcoded): B=16 graphs, N=2048 nodes, F=64 feats, K=4, out_dim=128.
  L = D A0 D  (A0 = A with zeroed diag, D = diag(1/(eps+sqrt(rowsum(A0)))))
  T0 = X; T1 = L X; T_t = 2 L T_{t-1} - T_{t-2}
  out = relu(concat(T0..T3) @ kernel + bias)

Sharding: batch across 8 cores, 2 graphs per core; host concatenates outputs.

Layout decisions:
- Host stages A TRANSPOSED and cast to fp8 e4m3. A loaded chunk holds
  At[j in chunk, i] with the contraction index j on partitions, so it is
  directly the PE's stationary operand: out[i-block, f] += sum_j A[i,j] Z[j,f]
  with the Z chunk as the moving operand (no transposes of A anywhere).
- fp8 A makes each [128, 2048] chunk cost ~790ns of queue time to load;
  chunks are round-robined over all 3 DMA queues (SP/ACT HWDGE + Pool SWDGE).
- Z is kept in TWO representations: Zp (bf16) feeds the projection and the
  recurrence subtraction; Zm (fp8, cast on the Pool engine) feeds the
  Chebyshev matmuls, which then run in DoubleRow perf mode (2 chunks of K
  per instruction at 0.5 cycles/row -> 4x the bf16 matmul throughput).
  Predicted end-to-end rel err 0.0145 (numpy model), gate is 2e-2.

Per core (graphs g=0,1), with Z_t := d * T_t:
  d from rowsums: column-sums of At chunks via 1-col ones matmuls on PE
  Z0      = d*X
  Z_t     = 2 d^2 * (A0 @ Z_{t-1}) - Z_{t-2}     (node-major throughout)
  out     = relu( (1/d) * (sum_t Z_t @ K_t) + bias )
The (1/d) row scale is folded into the e-scaled Z^T tiles used by the
projection (via a diag(e) moving operand in the transpose matmuls).
Graph 0's compute overlaps graph 1's load.
"""

import numpy as np

P = 128          # partitions
N = 2048         # nodes per graph
F = 64           # input features
KORD = 4         # Chebyshev order
OUT = 128        # output features
GP = 2           # graphs per core
NT = N // P      # 16 node chunks
NCORES = 8
CPD = 4          # max chunks per A dma_start
DOUBLE_ROW = True  # fp8 DoubleRow Chebyshev matmuls (vs bf16)

_cached = {}


def _build_nc():
    import ml_dtypes
    import concourse.bacc as bacc
    import concourse.mybir as mybir
    from concourse.tile import TileContext

    f32 = mybir.dt.float32
    bf16 = mybir.dt.bfloat16
    f8 = mybir.dt.float8e4
    Alu = mybir.AluOpType
    Act = mybir.ActivationFunctionType
    PM = mybir.MatmulPerfMode

    nc = bacc.Bacc("TRN2", target_bir_lowering=False)

    # A^T, pre-cast to fp8 e4m3 on the host.
    a_in = nc.dram_tensor("a", [GP, N, N], f8, kind="ExternalInput")
    x_in = nc.dram_tensor("x", [GP, N, F], f32, kind="ExternalInput")
    wk_in = nc.dram_tensor("wk", [KORD * F, OUT], f32, kind="ExternalInput")
    bias_in = nc.dram_tensor("bias", [OUT], f32, kind="ExternalInput")
    o_out = nc.dram_tensor("out", [GP, N, OUT], f32, kind="ExternalOutput")

    ident_np = np.eye(P, dtype=ml_dtypes.bfloat16)
    ident_dram = nc.inline_tensor(ident_np, name="identbf")

    with TileContext(nc) as tc, tc.tile_pool(name="const", bufs=1) as const, \
         tc.tile_pool(name="big", bufs=1) as big, \
         tc.tile_pool(name="small", bufs=1) as small, \
         tc.tile_pool(name="stage", bufs=2) as stage, \
         tc.tile_pool(name="outs", bufs=2) as outs, \
         tc.tile_pool(name="ps_rs", bufs=1, space="PSUM") as ps_rs, \
         tc.tile_pool(name="ps_it", bufs=5, space="PSUM") as ps_it, \
         tc.tile_pool(name="ps_ms", bufs=2, space="PSUM") as ps_ms:

        # ---- constants -------------------------------------------------
        ident = const.tile([P, P], bf16)
        nc.sync.dma_start(out=ident, in_=ident_dram[:, :])
        # mask = 1 - I in fp8 (for diag zeroing of the fp8 A tiles)
        mask = const.tile([P, P], f8)
        nc.vector.tensor_scalar(mask, ident, -1.0, 1.0, Alu.mult, Alu.add)
        # ones for the rowsum matmuls (fp8, exact)
        ones2 = const.tile([P, 2, 1], f8)
        nc.vector.memset(ones2, 1.0)
        # kernel: [256,128] f32 -> two bf16 tiles [128,128] (t-pairs 01, 23)
        kab = const.tile([P, OUT], bf16)
        kcd = const.tile([P, OUT], bf16)
        # bias row [1,128] bf16 + ones row [1,128] bf16 (for bias matmul)
        bias_row = const.tile([1, OUT], bf16)
        bias_f32 = const.tile([1, OUT], f32)
        ones_row = const.tile([1, P], bf16)
        nc.vector.memset(ones_row, 1.0)

        # ---- persistent SBUF state ------------------------------------
        # A^T per graph (fp8): [:, q, :] holds rows j = 128q+p, free = node i
        at = [big.tile([P, NT, N], f8, name=f"at{g}") for g in range(GP)]
        # Z bf16 (projection + recurrence): zja = (Z0|Z1), zjb = (Z2|Z3)
        zja = [big.tile([P, NT, 2 * F], bf16, name=f"zja{g}") for g in range(GP)]
        zjb = [big.tile([P, NT, 2 * F], bf16, name=f"zjb{g}") for g in range(GP)]
        # Z fp8 (matmul operand)
        zma = [big.tile([P, NT, 2 * F], f8, name=f"zma{g}") for g in range(GP)]
        zmb = [big.tile([P, NT, 2 * F], f8, name=f"zmb{g}") for g in range(GP)]
        # X staging (f32) per graph
        xst = [big.tile([P, NT, F], f32, name=f"xst{g}") for g in range(GP)]
        # e-scaled Z^T pairs per graph: rows 0:64 = even t, 64:128 = odd t
        ztab = [big.tile([P, N], bf16, name=f"ztab{g}") for g in range(GP)]
        ztcd = [big.tile([P, N], bf16, name=f"ztcd{g}") for g in range(GP)]
        # diag(e) moving operands for the Z^T builds
        dge = [big.tile([P, NT, P], bf16, name=f"dge{g}") for g in range(GP)]
        # row stats per graph [128, NT] f32 (col = node chunk)
        rs = [small.tile([P, NT], f32, name=f"rs{g}") for g in range(GP)]
        dd = [small.tile([P, NT], f32, name=f"dd{g}") for g in range(GP)]   # d/2
        d2s = [small.tile([P, NT], f32, name=f"d2s{g}") for g in range(GP)] # 2d^2
        evec = [small.tile([P, NT], f32, name=f"ev{g}") for g in range(GP)] # 1/d
        tch = [small.tile([P, NT], f32, name=f"tch{g}") for g in range(GP)]
        uch = [small.tile([P, NT], f32, name=f"uch{g}") for g in range(GP)]
        wch = [small.tile([P, NT], f32, name=f"wch{g}") for g in range(GP)]

        def zslice(g, t, q):
            base = zja[g] if t < 2 else zjb[g]
            h = t % 2
            return base[:, q, h * F : (h + 1) * F]

        def zmslice(g, t, q0, qn):
            base = zma[g] if t < 2 else zmb[g]
            h = t % 2
            return base[:, q0 : q0 + qn, h * F : (h + 1) * F]

        # rowsum PSUM: one tile, 16 cols per graph
        rsps = ps_rs.tile([P, GP * NT], f32, name="rsps")

        # ---- loads: A chunks round-robined over 3 queues; per-queue -----
        # cost ~ bytes/partition, so fp8 chunks are ~790ns each.
        lanes = [nc.sync, nc.scalar, nc.gpsimd]
        asgn = [0, 0, 0, 0, 0, 1, 1, 1, 1, 1, 2, 2, 2, 2, 2, 2]  # SP:5 ACT:5 Pool:6
        for g in range(GP):
            for lane in range(3):
                chunks = [q for q in range(NT) if asgn[q] == lane]
                for i0 in range(0, len(chunks), CPD):
                    grp = chunks[i0 : i0 + CPD]
                    c0, cn = grp[0], len(grp)
                    lanes[lane].dma_start(
                        out=at[g][:, c0 : c0 + cn, :],
                        in_=a_in[g, c0 * P : (c0 + cn) * P, :].rearrange(
                            "(c p) n -> p c n", p=P
                        ),
                    )
            # X after this graph's A chunks on the SP queue
            nc.sync.dma_start(
                out=xst[g],
                in_=x_in[g].rearrange("(c p) f -> p c f", p=P),
            )
        # projection constants load late on SP (needed only at proj time)
        kstage = stage.tile([P, 2 * OUT], f32, name="kstage")
        nc.sync.dma_start(out=kstage[:, 0:OUT], in_=wk_in[0:P, :])
        nc.sync.dma_start(out=kstage[:, OUT : 2 * OUT], in_=wk_in[P : 2 * P, :])
        nc.scalar.copy(out=kab, in_=kstage[:, 0:OUT])
        nc.scalar.copy(out=kcd, in_=kstage[:, OUT : 2 * OUT])
        nc.sync.dma_start(out=bias_f32, in_=bias_in[None, :])
        nc.scalar.copy(out=bias_row, in_=bias_f32)

        def prep(g):
            # zero diag blocks (chase chunk arrivals)
            for q in range(NT):
                blk = at[g][:, q, q * P : (q + 1) * P]
                nc.vector.scalar_tensor_tensor(blk, blk, 1.0, mask, Alu.mult, Alu.mult)
            # rowsums: d_i = sum_j A0[i,j]. Sequential groups per psum tile;
            # DoubleRow over chunk pairs (ones moving operand, 1 col out).
            for b in range(NT):
                col = g * NT + b
                for q2 in range(NT // 2):
                    nc.tensor.matmul(
                        rsps[:, col : col + 1],
                        lhsT=at[g][:, 2 * q2 : 2 * q2 + 2, b * P : (b + 1) * P],
                        rhs=ones2,
                        start=(q2 == 0), stop=(q2 == NT // 2 - 1),
                        perf_mode=PM.DoubleRow,
                        skip_group_check=True,
                    )
            nc.vector.tensor_copy(rs[g], rsps[:, g * NT : (g + 1) * NT])
            # d chain (Newton-refined sqrt: w = rs/sqrt(rs) + sqrt(rs) = 2 sqrt)
            nc.scalar.activation(tch[g], rs[g], Act.Sqrt)
            nc.vector.reciprocal(uch[g], tch[g])
            nc.vector.scalar_tensor_tensor(uch[g], uch[g], 1.0, rs[g], Alu.mult, Alu.mult)
            nc.vector.scalar_tensor_tensor(wch[g], uch[g], 1.0, tch[g], Alu.mult, Alu.add)
            # dd = 1/w = d/2 ; e = w/2 = 1/d ; d2s = 8*dd^2 = 2 d^2
            nc.vector.reciprocal(dd[g], wch[g])
            nc.vector.tensor_scalar_mul(evec[g], wch[g], 0.5)
            nc.vector.scalar_tensor_tensor(d2s[g], dd[g], 8.0, dd[g], Alu.mult, Alu.mult)
            # Z0 = d*X = (X*dd)*2 ; fp8 copy on Pool
            for c in range(NT):
                nc.vector.tensor_scalar(
                    zslice(g, 0, c), xst[g][:, c, :],
                    dd[g][:, c : c + 1], 2.0, Alu.mult, Alu.mult,
                )
                if DOUBLE_ROW:
                    nc.gpsimd.tensor_copy(zmslice(g, 0, c, 1), zslice(g, 0, c))
            # diag(e) moving operands (needed from ztpair builds on)
            for c in range(NT):
                nc.vector.tensor_scalar_mul(
                    dge[g][:, c, :], ident, evec[g][:, c : c + 1]
                )

        def iteration(g, t):
            # Z_t = 2 d^2 (A0 @ Z_{t-1}) - Z_{t-2}, 16 i-blocks, psum in halves
            for half in range(2):
                ps = ps_it.tile([P, 512], f32, name="psit", tag="psit")
                for bb in range(8):
                    b = half * 8 + bb
                    reg = ps[:, bb * F : (bb + 1) * F]
                    if DOUBLE_ROW:
                        for q2 in range(NT // 2):
                            nc.tensor.matmul(
                                reg,
                                lhsT=at[g][:, 2 * q2 : 2 * q2 + 2, b * P : (b + 1) * P],
                                rhs=zmslice(g, t - 1, 2 * q2, 2),
                                start=(q2 == 0), stop=(q2 == NT // 2 - 1),
                                perf_mode=PM.DoubleRow,
                                skip_group_check=True,
                            )
                    else:
                        for q in range(NT):
                            nc.tensor.matmul(
                                reg,
                                lhsT=at[g][:, q, b * P : (b + 1) * P],
                                rhs=zslice(g, t - 1, q),
                                start=(q == 0), stop=(q == NT - 1),
                                skip_group_check=True,
                            )
                    zdst = zslice(g, t, b)
                    if t == 1:
                        # Z1 = d^2 W = (W * d2s) * 0.5
                        nc.vector.tensor_scalar(
                            zdst, reg, d2s[g][:, b : b + 1], 0.5,
                            Alu.mult, Alu.mult,
                        )
                    else:
                        nc.vector.scalar_tensor_tensor(
                            zdst, reg, d2s[g][:, b : b + 1],
                            zslice(g, t - 2, b), Alu.mult, Alu.subtract,
                        )
                    if DOUBLE_ROW and t < KORD - 1:
                        nc.gpsimd.tensor_copy(zmslice(g, t, b, 1), zdst)

        def ztpair(g, pair):
            # e-scaled transposed copies: ztab (T0|T1) or ztcd (T2|T3)
            src = zja[g] if pair == 0 else zjb[g]
            dst = ztab[g] if pair == 0 else ztcd[g]
            for s in range(4):
                psz = ps_ms.tile([P, 512], f32, name="psz", tag="psms")
                for j in range(4):
                    c = 4 * s + j
                    nc.tensor.matmul(
                        psz[:, j * P : (j + 1) * P],
                        lhsT=src[:, c, :],
                        rhs=dge[g][:, c, :],
                        start=True, stop=True,
                        skip_group_check=True,
                    )
                nc.scalar.copy(out=dst[:, s * 512 : (s + 1) * 512], in_=psz)

        def proj(g):
            for s in range(4):
                pso = ps_ms.tile([P, 512], f32, name="pso", tag="psms")
                for j in range(4):
                    b = 4 * s + j
                    sl = pso[:, j * OUT : (j + 1) * OUT]
                    nc.tensor.matmul(
                        sl, lhsT=ztab[g][:, b * P : (b + 1) * P], rhs=kab,
                        start=True, stop=False, skip_group_check=True,
                    )
                    nc.tensor.matmul(
                        sl, lhsT=ztcd[g][:, b * P : (b + 1) * P], rhs=kcd,
                        start=False, stop=False, skip_group_check=True,
                    )
                    nc.tensor.matmul(
                        sl, lhsT=ones_row, rhs=bias_row,
                        start=False, stop=True, skip_group_check=True,
                    )
                ot = outs.tile([P, 4, OUT], f32, name="ot", tag="ot")
                nc.scalar.activation(ot, pso, Act.Relu)
                nc.sync.dma_start(
                    out=o_out[g, s * 512 : (s + 1) * 512, :].rearrange(
                        "(j p) o -> p j o", p=P
                    ),
                    in_=ot,
                )

        # ---- schedule: g0 compute overlaps g1 load --------------------
        prep(0)
        iteration(0, 1)
        prep(1)
        ztpair(0, 0)
        iteration(0, 2)
        iteration(0, 3)
        ztpair(0, 1)
        proj(0)
        iteration(1, 1)
        ztpair(1, 0)
        iteration(1, 2)
        iteration(1, 3)
        ztpair(1, 1)
        proj(1)

    nc.finalize()
    return nc


def _get_nc():
    if "nc" not in _cached:
        _cached["nc"] = _build_nc()
    return _cached["nc"]


def kernel(X, A, kernel, bias):
    import ml_dtypes
    from concourse.bass_utils import run_bass_kernel_spmd

    nc = _get_nc()
    wk = np.ascontiguousarray(np.asarray(kernel, dtype=np.float32))
    bs = np.ascontiguousarray(np.asarray(bias, dtype=np.float32))
    A = np.asarray(A, dtype=np.float32)
    X = np.asarray(X, dtype=np.float32)
    in_maps = [
        {
            # stage A transposed (device contracts over partitions = A
            # columns) and fp8 e4m3 (the dtype the device kernel computes in)
            "a": np.ascontiguousarray(
                A[GP * c : GP * (c + 1)].transpose(0, 2, 1)
            ).astype(ml_dtypes.float8_e4m3),
            "x": np.ascontiguousarray(X[GP * c : GP * (c + 1)]),
            "wk": wk,
            "bias": bs,
        }
        for c in range(NCORES)
    ]
    res = run_bass_kernel_spmd(nc, in_maps, core_ids=list(range(NCORES)))
    return np.concatenate([r["out"] for r in res.results], axis=0)
